# revision 22
# baseline (speedup 1.0000x reference)
"""Butterworth IIR (order 4) over [B=128, T=160000, 1] on 8 TRN2 NeuronCores.

Strategy: a stable IIR's impulse response decays geometrically (max pole
radius ~0.668 here), so the filter is numerically exactly (tail < 3e-23)
a 128-tap causal FIR:  y[t] = sum_{k<128} h[k] x[t-k].

Chunking time into 128-sample chunks, with X[c, m] = x[c*128 + m]:
    y[c*128 + j] = sum_m X[c, m] H0[m, j] + sum_m X[c-1, m] H1[m, j]
    H0[m, j] = h[j - m]        (0 <= j - m < 128)
    H1[m, j] = h[j - m + 128]  (0 <= j - m + 128 < 128)

On device this is two accumulating TensorE matmuls per window with the
small fixed H matrices as the stationary operand and a phase-major
(transposed) view of x as the wide moving operand (N up to 512 chunks).

v2 (DMA-roofline focus — the kernel is HBM-bound at ~358 GB/s/core):
  * f16 output instead of f32: total HBM traffic 15.4MB -> 10.3MB/core.
    (f16 rounding adds ~3e-4 rel err; gate is 2e-2.)
  * All 16 sequences' inputs live in one SBUF tile (5.1MB), with all
    input DMAs issued up front on the sync queue so the HBM pipe never
    starves; outputs go on the gpsimd (SWDGE) queue.
  * PSUM evacuation (with the f32->f16 cast) split between the scalar
    (ACT) and vector (DVE) engines so neither is the bottleneck.
  * Matmuls grouped per sequence: H0 over all 3 windows, then H1,
    to keep the PE array streaming with minimal weight switching.

Sharding: pure data-parallel, batch 128 -> 16 sequences per core.
"""

import numpy as np

B_FULL = 128
T_FULL = 160000
N_CORES = 8
SEQ_PER_CORE = B_FULL // N_CORES  # 16
CHUNK = 128
NCHUNK = T_FULL // CHUNK  # 1250
TAPS = 128
NWIN = 417  # matmul moving-operand width; 3 equal-ish windows (417/417/416)
            # keep every MM long enough that the next LDWEIGHTS fully hides
WARMUP_MM = 8  # dummy matmuls on the H tile to pull the HAM 2.4GHz
               # transition ahead of the first real matmul
SEQ_COLS = NCHUNK + 1  # 1251: col 0 is the zero predecessor chunk

_NC_CACHE = {}


def _impulse_response(b, a, n):
    """First n samples of the IIR impulse response, computed in float64
    via the same direct-form II transposed recurrence as the reference."""
    b = np.asarray(b, np.float64)
    a = np.asarray(a, np.float64)
    bn = b / a[0]
    an = a / a[0]
    order = len(a) - 1
    z = np.zeros(order, np.float64)
    h = np.zeros(n, np.float64)
    xt = 1.0
    for t in range(n):
        yt = bn[0] * xt + z[0]
        znew = np.empty_like(z)
        znew[:-1] = z[1:] + xt * bn[1:-1] - yt * an[1:-1]
        znew[-1] = xt * bn[-1] - yt * an[-1]
        z = znew
        h[t] = yt
        xt = 0.0
    return h


def _build_h_matrices(b, a):
    h = _impulse_response(b, a, TAPS)
    m = np.arange(CHUNK)[:, None]
    j = np.arange(CHUNK)[None, :]
    d0 = j - m
    d1 = j - m + CHUNK
    H0 = np.where((d0 >= 0) & (d0 < TAPS), h[np.clip(d0, 0, TAPS - 1)], 0.0)
    H1 = np.where((d1 >= 0) & (d1 < TAPS), h[np.clip(d1, 0, TAPS - 1)], 0.0)
    return np.concatenate([H0, H1], axis=1).astype(np.float16)  # [128, 256]


def _build_nc():
    import concourse.bacc as bacc
    import concourse.mybir as mybir
    from concourse.tile import TileContext

    f32 = mybir.dt.float32
    f16 = mybir.dt.float16
    nc = bacc.Bacc()
    xt = nc.declare_dram_parameter(
        "xt", [CHUNK, SEQ_PER_CORE * SEQ_COLS], f16, isOutput=False
    )
    hh = nc.declare_dram_parameter("hh", [CHUNK, 2 * CHUNK], f16, isOutput=False)
    yt = nc.declare_dram_parameter(
        "yt", [CHUNK, SEQ_PER_CORE * NCHUNK], f16, isOutput=True
    )

    wins = list(range(0, NCHUNK, NWIN))  # [0, 512, 1024]

    with TileContext(nc) as tc:
        with (
            tc.tile_pool(name="const", bufs=1) as cpool,
            tc.tile_pool(name="yout", bufs=SEQ_PER_CORE) as ypool,
            tc.tile_pool(name="acc", bufs=7, space="PSUM") as pspool,
        ):
            h_tile = cpool.tile([CHUNK, 2 * CHUNK], f16)
            nc.sync.dma_start(out=h_tile[:], in_=hh[:])
            # one resident tile holds every sequence's phase-major input;
            # all input DMAs are issued up front, split across the two
            # HWDGE queues (sync & scalar) so the read side of the HBM
            # pipe is never issue- or queue-starved. seq 0 lands in
            # window-sized pieces so the first matmul starts ASAP.
            x_tile = cpool.tile([CHUNK, SEQ_PER_CORE * SEQ_COLS], f16)
            cuts = [0] + [w + min(NWIN, NCHUNK - w) + 1 for w in wins]  # non-overlapping
            for lo, hi in zip(cuts[:-1], cuts[1:]):
                nc.scalar.dma_start(out=x_tile[:, lo:hi], in_=xt[:, lo:hi])
            for s in range(1, SEQ_PER_CORE):
                lo, hi = s * SEQ_COLS, (s + 1) * SEQ_COLS
                eng = nc.sync if s < 8 else nc.scalar
                eng.dma_start(out=x_tile[:, lo:hi], in_=xt[:, lo:hi])

            # dummy matmuls on the already-resident H tile: keeps the PE
            # array continuously busy from the moment H lands, so the HAM
            # clock gate reaches 2.4GHz ~7us earlier (it needs ~3.4us of
            # sustained activity). Results go to a scratch PSUM bank and
            # are never read.
            warm_ps = pspool.tile([CHUNK, NWIN], f32, bufs=1)
            for _ in range(WARMUP_MM):
                nc.tensor.matmul(
                    warm_ps[:, : 2 * CHUNK],
                    h_tile[:, 0:CHUNK],
                    h_tile[:],
                    start=True,
                    stop=True,
                )

            pending_out = []
            for s in range(SEQ_PER_CORE):
                base = s * SEQ_COLS
                y_tile = ypool.tile([CHUNK, NCHUNK], f16)
                ps = []
                for w in wins:
                    n = min(NWIN, NCHUNK - w)
                    p = pspool.tile([CHUNK, NWIN], f32)
                    nc.tensor.matmul(
                        p[:, :n],
                        h_tile[:, 0:CHUNK],
                        x_tile[:, base + w + 1 : base + w + 1 + n],
                        start=True,
                        stop=False,
                    )
                    ps.append(p)
                for i, w in enumerate(wins):
                    n = min(NWIN, NCHUNK - w)
                    nc.tensor.matmul(
                        ps[i][:, :n],
                        h_tile[:, CHUNK : 2 * CHUNK],
                        x_tile[:, base + w : base + w + n],
                        start=False,
                        stop=True,
                    )
                # evacuate PSUM (f32) to SBUF as f16, balanced across the
                # ACT and DVE engines: ACT takes w0 + the first half of w2,
                # DVE takes w1 + the second half of w2.
                w0, w1, w2 = wins
                n2 = NCHUNK - w2
                h2 = n2 // 2
                nc.scalar.copy(out=y_tile[:, w0 : w0 + NWIN], in_=ps[0][:, :NWIN])
                nc.vector.tensor_copy(
                    out=y_tile[:, w1 : w1 + NWIN], in_=ps[1][:, :NWIN]
                )
                nc.scalar.copy(out=y_tile[:, w2 : w2 + h2], in_=ps[2][:, :h2])
                nc.vector.tensor_copy(
                    out=y_tile[:, w2 + h2 : w2 + n2], in_=ps[2][:, h2:n2]
                )
                while pending_out:
                    eng, dst_lo, dst_hi, tile = pending_out.pop()
                    eng.dma_start(out=yt[:, dst_lo:dst_hi], in_=tile[:])
                # first 12 sequences alternate the gpsimd/sync queues; the
                # last four also use the (by then input-free) scalar queue.
                # scalar-bound outputs are emitted one sequence late so
                # their semaphore waits can't stall the ACT copies.
                out_eng = (
                    (nc.gpsimd if s % 2 == 0 else nc.sync)
                    if s < 12
                    else [nc.scalar, nc.sync, nc.gpsimd, nc.scalar][s - 12]
                )
                if out_eng is nc.scalar and s < SEQ_PER_CORE - 1:
                    pending_out.append(
                        (out_eng, s * NCHUNK, (s + 1) * NCHUNK, y_tile)
                    )
                else:
                    out_eng.dma_start(
                        out=yt[:, s * NCHUNK : (s + 1) * NCHUNK], in_=y_tile[:]
                    )
    nc.compile()
    return nc


def _run_on_device(in_maps, trace=False):
    from concourse.bass_utils import run_bass_kernel_spmd

    if "nc" not in _NC_CACHE:
        _NC_CACHE["nc"] = _build_nc()
    return run_bass_kernel_spmd(
        _NC_CACHE["nc"], in_maps, core_ids=list(range(N_CORES)), trace=trace
    )


def _prepare_in_maps(x, b, a):
    hh = _build_h_matrices(b, a)
    xs = np.ascontiguousarray(np.asarray(x, np.float32).reshape(B_FULL, T_FULL))
    in_maps = []
    for c in range(N_CORES):
        xc = xs[c * SEQ_PER_CORE : (c + 1) * SEQ_PER_CORE]
        # phase-major: xt[p, s*1251 + 1 + c'] = x[s, c'*128 + p]; col 0 of
        # each sequence block is zeros (the "previous chunk" of chunk 0).
        xt = np.zeros((CHUNK, SEQ_PER_CORE, SEQ_COLS), np.float16)
        xt[:, :, 1:] = xc.reshape(SEQ_PER_CORE, NCHUNK, CHUNK).transpose(2, 0, 1)
        in_maps.append({"xt": np.ascontiguousarray(xt.reshape(CHUNK, -1)), "hh": hh})
    return in_maps


def _assemble_output(results):
    out = np.empty((B_FULL, T_FULL, 1), np.float32)
    for c in range(N_CORES):
        ytc = np.asarray(results[c]["yt"]).reshape(CHUNK, SEQ_PER_CORE, NCHUNK)
        yc = ytc.transpose(1, 2, 0).reshape(SEQ_PER_CORE, T_FULL)
        out[c * SEQ_PER_CORE : (c + 1) * SEQ_PER_CORE, :, 0] = yc.astype(np.float32)
    return out


def kernel(x, b, a):
    in_maps = _prepare_in_maps(x, b, a)
    res = _run_on_device(in_maps, trace=False)
    return _assemble_output(res.results)


def kernel_traced(x, b, a):
    """Same as kernel() but with neuron profiling; returns (output, exec_time_ns)."""
    in_maps = _prepare_in_maps(x, b, a)
    try:
        res = _run_on_device(in_maps, trace=True)
    except ModuleNotFoundError:
        res = _run_on_device(in_maps, trace=False)
    return _assemble_output(res.results), res.exec_time_ns


# revision 23
# speedup vs baseline: 1.0365x; 1.0365x over previous
"""Butterworth IIR (order 4) over [B=128, T=160000, 1] on 8 TRN2 NeuronCores.

Strategy: a stable IIR's impulse response decays geometrically (max pole
radius ~0.668 here), so the filter is numerically exactly (tail < 3e-23)
a 128-tap causal FIR:  y[t] = sum_{k<128} h[k] x[t-k].

Chunking time into 128-sample chunks, with X[c, m] = x[c*128 + m]:
    y[c*128 + j] = sum_m X[c, m] H0[m, j] + sum_m X[c-1, m] H1[m, j]
    H0[m, j] = h[j - m]        (0 <= j - m < 128)
    H1[m, j] = h[j - m + 128]  (0 <= j - m + 128 < 128)

On device this is two accumulating TensorE matmuls per window with the
small fixed H matrices as the stationary operand and a phase-major
(transposed) view of x as the wide moving operand (N up to 512 chunks).

v2 (DMA-roofline focus — the kernel is HBM-bound at ~358 GB/s/core):
  * f16 output instead of f32: total HBM traffic 15.4MB -> 10.3MB/core.
    (f16 rounding adds ~3e-4 rel err; gate is 2e-2.)
  * All 16 sequences' inputs live in one SBUF tile (5.1MB), with all
    input DMAs issued up front on the sync queue so the HBM pipe never
    starves; outputs go on the gpsimd (SWDGE) queue.
  * PSUM evacuation (with the f32->f16 cast) split between the scalar
    (ACT) and vector (DVE) engines so neither is the bottleneck.
  * Matmuls grouped per sequence: H0 over all 3 windows, then H1,
    to keep the PE array streaming with minimal weight switching.

Sharding: pure data-parallel, batch 128 -> 16 sequences per core.
"""

import numpy as np

B_FULL = 128
T_FULL = 160000
N_CORES = 8
SEQ_PER_CORE = B_FULL // N_CORES  # 16
CHUNK = 128
NCHUNK = T_FULL // CHUNK  # 1250
TAPS = 128
NWIN = 417  # matmul moving-operand width; 3 equal-ish windows (417/417/416)
            # keep every MM long enough that the next LDWEIGHTS fully hides
WARMUP_MM = 8  # dummy matmuls on the H tile to pull the HAM 2.4GHz
               # transition ahead of the first real matmul
SEQ_COLS = NCHUNK + 1  # 1251: col 0 is the zero predecessor chunk

_NC_CACHE = {}


def _impulse_response(b, a, n):
    """First n samples of the IIR impulse response, computed in float64
    via the same direct-form II transposed recurrence as the reference."""
    b = np.asarray(b, np.float64)
    a = np.asarray(a, np.float64)
    bn = b / a[0]
    an = a / a[0]
    order = len(a) - 1
    z = np.zeros(order, np.float64)
    h = np.zeros(n, np.float64)
    xt = 1.0
    for t in range(n):
        yt = bn[0] * xt + z[0]
        znew = np.empty_like(z)
        znew[:-1] = z[1:] + xt * bn[1:-1] - yt * an[1:-1]
        znew[-1] = xt * bn[-1] - yt * an[-1]
        z = znew
        h[t] = yt
        xt = 0.0
    return h


def _build_h_matrices(b, a):
    h = _impulse_response(b, a, TAPS)
    m = np.arange(CHUNK)[:, None]
    j = np.arange(CHUNK)[None, :]
    d0 = j - m
    d1 = j - m + CHUNK
    H0 = np.where((d0 >= 0) & (d0 < TAPS), h[np.clip(d0, 0, TAPS - 1)], 0.0)
    H1 = np.where((d1 >= 0) & (d1 < TAPS), h[np.clip(d1, 0, TAPS - 1)], 0.0)
    return np.concatenate([H0, H1], axis=1).astype(np.float16)  # [128, 256]


def _build_nc():
    import concourse.bacc as bacc
    import concourse.mybir as mybir
    from concourse.tile import TileContext

    f32 = mybir.dt.float32
    f16 = mybir.dt.float16
    nc = bacc.Bacc()
    xt = nc.declare_dram_parameter(
        "xt", [CHUNK, SEQ_PER_CORE * SEQ_COLS], f16, isOutput=False
    )
    hh = nc.declare_dram_parameter("hh", [CHUNK, 2 * CHUNK], f16, isOutput=False)
    yt = nc.declare_dram_parameter(
        "yt", [CHUNK, SEQ_PER_CORE * NCHUNK], f16, isOutput=True
    )

    wins = list(range(0, NCHUNK, NWIN))  # [0, 512, 1024]

    with TileContext(nc) as tc:
        with (
            tc.tile_pool(name="const", bufs=1) as cpool,
            tc.tile_pool(name="yout", bufs=SEQ_PER_CORE) as ypool,
            tc.tile_pool(name="acc", bufs=7, space="PSUM") as pspool,
        ):
            h_tile = cpool.tile([CHUNK, 2 * CHUNK], f16)
            nc.sync.dma_start(out=h_tile[:], in_=hh[:])
            # one resident tile holds every sequence's phase-major input;
            # all input DMAs are issued up front, split across the two
            # HWDGE queues (sync & scalar) so the read side of the HBM
            # pipe is never issue- or queue-starved. seq 0 lands in
            # window-sized pieces so the first matmul starts ASAP.
            x_tile = cpool.tile([CHUNK, SEQ_PER_CORE * SEQ_COLS], f16)
            # seq0's window pieces split across both HWDGE queues so the
            # first matmul can start as early as possible; later sequences
            # alternate queues in need order.
            cuts = [0] + [w + min(NWIN, NCHUNK - w) + 1 for w in wins]  # non-overlapping
            piece_engs = [nc.sync, nc.scalar, nc.sync]
            for eng, (lo, hi) in zip(piece_engs, zip(cuts[:-1], cuts[1:])):
                eng.dma_start(out=x_tile[:, lo:hi], in_=xt[:, lo:hi])
            for s in range(1, SEQ_PER_CORE):
                lo, hi = s * SEQ_COLS, (s + 1) * SEQ_COLS
                eng = nc.scalar if s % 2 == 1 else nc.sync
                eng.dma_start(out=x_tile[:, lo:hi], in_=xt[:, lo:hi])

            # Filler matmuls on the already-resident H tile keep the PE
            # array busy through the input-limited ramp: the HAM clock
            # gate needs ~3.4us of sustained activity to unlock 2.4GHz,
            # and any multi-us idle gap resets it. Results land in a
            # scratch PSUM bank and are never read.
            warm_ps = pspool.tile([CHUNK, NWIN], f32, bufs=1)

            def filler(n_mm, cols=CHUNK):
                for _ in range(n_mm):
                    nc.tensor.matmul(
                        warm_ps[:, :cols],
                        h_tile[:, 0:CHUNK],
                        h_tile[:, :cols],
                        start=True,
                        stop=True,
                    )

            filler(6, 2 * CHUNK)

            pending_out = []
            for s in range(SEQ_PER_CORE):
                base = s * SEQ_COLS
                y_tile = ypool.tile([CHUNK, NCHUNK], f16)
                ps = []
                for w in wins:
                    n = min(NWIN, NCHUNK - w)
                    p = pspool.tile([CHUNK, NWIN], f32)
                    nc.tensor.matmul(
                        p[:, :n],
                        h_tile[:, 0:CHUNK],
                        x_tile[:, base + w + 1 : base + w + 1 + n],
                        start=True,
                        stop=False,
                    )
                    if s < 3:
                        filler(3)
                    ps.append(p)
                for i, w in enumerate(wins):
                    n = min(NWIN, NCHUNK - w)
                    nc.tensor.matmul(
                        ps[i][:, :n],
                        h_tile[:, CHUNK : 2 * CHUNK],
                        x_tile[:, base + w : base + w + n],
                        start=False,
                        stop=True,
                    )
                    if s < 3:
                        filler(3)
                # evacuate PSUM (f32) to SBUF as f16, balanced across the
                # ACT and DVE engines: ACT takes w0 + the first half of w2,
                # DVE takes w1 + the second half of w2.
                w0, w1, w2 = wins
                n2 = NCHUNK - w2
                h2 = n2 // 2
                nc.scalar.copy(out=y_tile[:, w0 : w0 + NWIN], in_=ps[0][:, :NWIN])
                nc.vector.tensor_copy(
                    out=y_tile[:, w1 : w1 + NWIN], in_=ps[1][:, :NWIN]
                )
                nc.scalar.copy(out=y_tile[:, w2 : w2 + h2], in_=ps[2][:, :h2])
                nc.vector.tensor_copy(
                    out=y_tile[:, w2 + h2 : w2 + n2], in_=ps[2][:, h2:n2]
                )
                while pending_out:
                    eng, dst_lo, dst_hi, tile = pending_out.pop()
                    eng.dma_start(out=yt[:, dst_lo:dst_hi], in_=tile[:])
                # first 12 sequences alternate the gpsimd/sync queues; the
                # last four also use the (by then input-free) scalar queue.
                # scalar-bound outputs are emitted one sequence late so
                # their semaphore waits can't stall the ACT copies.
                out_eng = (
                    (nc.gpsimd if s % 2 == 0 else nc.sync)
                    if s < 12
                    else [nc.scalar, nc.sync, nc.gpsimd, nc.scalar][s - 12]
                )
                if out_eng is nc.scalar and s < SEQ_PER_CORE - 1:
                    pending_out.append(
                        (out_eng, s * NCHUNK, (s + 1) * NCHUNK, y_tile)
                    )
                else:
                    out_eng.dma_start(
                        out=yt[:, s * NCHUNK : (s + 1) * NCHUNK], in_=y_tile[:]
                    )
    nc.compile()
    return nc


def _run_on_device(in_maps, trace=False):
    from concourse.bass_utils import run_bass_kernel_spmd

    if "nc" not in _NC_CACHE:
        _NC_CACHE["nc"] = _build_nc()
    return run_bass_kernel_spmd(
        _NC_CACHE["nc"], in_maps, core_ids=list(range(N_CORES)), trace=trace
    )


def _prepare_in_maps(x, b, a):
    hh = _build_h_matrices(b, a)
    xs = np.ascontiguousarray(np.asarray(x, np.float32).reshape(B_FULL, T_FULL))
    in_maps = []
    for c in range(N_CORES):
        xc = xs[c * SEQ_PER_CORE : (c + 1) * SEQ_PER_CORE]
        # phase-major: xt[p, s*1251 + 1 + c'] = x[s, c'*128 + p]; col 0 of
        # each sequence block is zeros (the "previous chunk" of chunk 0).
        xt = np.zeros((CHUNK, SEQ_PER_CORE, SEQ_COLS), np.float16)
        xt[:, :, 1:] = xc.reshape(SEQ_PER_CORE, NCHUNK, CHUNK).transpose(2, 0, 1)
        in_maps.append({"xt": np.ascontiguousarray(xt.reshape(CHUNK, -1)), "hh": hh})
    return in_maps


def _assemble_output(results):
    out = np.empty((B_FULL, T_FULL, 1), np.float32)
    for c in range(N_CORES):
        ytc = np.asarray(results[c]["yt"]).reshape(CHUNK, SEQ_PER_CORE, NCHUNK)
        yc = ytc.transpose(1, 2, 0).reshape(SEQ_PER_CORE, T_FULL)
        out[c * SEQ_PER_CORE : (c + 1) * SEQ_PER_CORE, :, 0] = yc.astype(np.float32)
    return out


def kernel(x, b, a):
    in_maps = _prepare_in_maps(x, b, a)
    res = _run_on_device(in_maps, trace=False)
    return _assemble_output(res.results)


def kernel_traced(x, b, a):
    """Same as kernel() but with neuron profiling; returns (output, exec_time_ns)."""
    in_maps = _prepare_in_maps(x, b, a)
    try:
        res = _run_on_device(in_maps, trace=True)
    except ModuleNotFoundError:
        res = _run_on_device(in_maps, trace=False)
    return _assemble_output(res.results), res.exec_time_ns
